# revision 1
# baseline (speedup 1.0000x reference)
"""Causal single-head attention on 8 TRN2 NeuronCores.

Problem: x [4, 4096, 1024] fp32, Wq/Wk/Wv [1024, 1024] fp32.
  q,k,v = x@W*;  out = softmax(mask(q@k^T)/sqrt(1024)) @ v   per batch.

Sharding: 2 cores per batch (4 batches x 2 = 8 cores). The two cores of a
batch split the KEY dimension by 128-key-tile parity: core h in {0,1} owns
key tiles {h, h+2, h+4, ...}. Every core processes all 4096 queries of its
batch against its ~half of the keys, producing unnormalized partial outputs
  O_h = sum_k exp(s_qk/32) v_k   and   l_h = sum_k exp(s_qk/32)
which the host combines as O = (O_0 + O_1) / (l_0 + l_1).

This parity split makes the per-core program *identical* (SPMD-friendly):
for query block Qb (256 queries = 2 query tiles), both parities process
exactly Qb+1 packed key tiles; the final packed tile is the "diagonal" tile
for one of the parities and either fully-allowed or fully-masked for the
other, handled by one per-core [128, 256] multiplicative mask.

On-device compute uses fp16 matmul inputs (fp32 PSUM accumulation):
fp16 keeps ~10 mantissa bits vs bf16's 8 at identical TensorE throughput.
Softmax skips max-subtraction: logits are ~N(0,1) for this distribution so
exp stays well within fp16/fp32 range (softmax is shift-invariant, so the
result is mathematically identical).
"""

import numpy as np

B, S, D = 4, 4096, 1024
N_CORES = 8
QB = 256            # queries per attention block (2 query tiles)
NQB = S // QB       # 16 blocks
SP = S // 2         # packed keys per core
NKT = SP // 128     # 16 packed key tiles per core
SCALE = 1.0 / 32.0  # 1/sqrt(D_out)

_PROGRAM_CACHE = {}


def _build_program():
    import concourse.mybir as mybir
    import concourse.tile as tile
    from concourse import bacc

    f16 = mybir.dt.float16
    f32 = mybir.dt.float32
    Exp = mybir.ActivationFunctionType.Exp

    nc = bacc.Bacc("TRN2", target_bir_lowering=False, debug=False,
                   num_devices=N_CORES)

    xT = nc.dram_tensor("xT", [D, S], f16, kind="ExternalInput").ap()
    xTp = nc.dram_tensor("xTp", [D, SP], f16, kind="ExternalInput").ap()
    wq = nc.dram_tensor("wq", [D, D], f16, kind="ExternalInput").ap()
    wk = nc.dram_tensor("wk", [D, D], f16, kind="ExternalInput").ap()
    wv = nc.dram_tensor("wv", [D, D], f16, kind="ExternalInput").ap()
    mask = nc.dram_tensor("mask", [128, QB], f16, kind="ExternalInput").ap()
    O = nc.dram_tensor("O", [S, D], f32, kind="ExternalOutput").ap()
    L = nc.dram_tensor("L", [1, S], f32, kind="ExternalOutput").ap()

    with tile.TileContext(nc) as tc:
        with tc.tile_pool(name="res", bufs=1) as res:
            # SBUF-resident projection outputs (layouts: partition x free)
            # kT: K^T packed; d-chunk c lives at cols [c*SP, (c+1)*SP)
            kT = res.tile([128, 8 * SP], f16, tag="kT")
            # v: packed V; key tile j at cols [j*D, (j+1)*D)
            v = res.tile([128, NKT * D], f16, tag="v")
            # qT: Q^T; d-chunk c at cols [c*S, (c+1)*S)
            qT = res.tile([128, 8 * S], f16, tag="qT")
            mask_sb = res.tile([128, QB], f16, tag="mask_sb")
            ones_sb = res.tile([128, 1], f16, tag="ones_sb")
            nc.sync.dma_start(mask_sb[:], mask[:, :])
            nc.vector.memset(ones_sb[:], 1.0)

            # ---------------- QKV projections ----------------
            with tc.tile_pool(name="w", bufs=1) as wpool, \
                 tc.tile_pool(name="xc", bufs=2) as xpool, \
                 tc.tile_pool(name="pproj", bufs=4, space="PSUM") as ppool:
                # W layout: d_in chunk c at cols [c*D, (c+1)*D)
                wk_sb = wpool.tile([128, 8 * D], f16, tag="w0")
                wv_sb = wpool.tile([128, 8 * D], f16, tag="w1")
                for c in range(8):
                    nc.sync.dma_start(wk_sb[:, c * D:(c + 1) * D],
                                      wk[c * 128:(c + 1) * 128, :])
                    nc.sync.dma_start(wv_sb[:, c * D:(c + 1) * D],
                                      wv[c * 128:(c + 1) * 128, :])

                # K^T and V from packed x^T, 512 packed keys per chunk
                for ci in range(SP // 512):
                    xc = xpool.tile([128, 8 * 512], f16, tag="xc")
                    for c in range(8):
                        nc.sync.dma_start(
                            xc[:, c * 512:(c + 1) * 512],
                            xTp[c * 128:(c + 1) * 128, ci * 512:(ci + 1) * 512])
                    for m in range(8):
                        pp = ppool.tile([128, 512], f32, tag="pp", name="pp")
                        for c in range(8):
                            nc.tensor.matmul(
                                pp[:],
                                wk_sb[:, c * D + m * 128: c * D + (m + 1) * 128],
                                xc[:, c * 512:(c + 1) * 512],
                                start=(c == 0), stop=(c == 7))
                        dst = kT[:, m * SP + ci * 512: m * SP + (ci + 1) * 512]
                        if m % 2 == 0:
                            nc.vector.tensor_copy(dst, pp[:])
                        else:
                            nc.scalar.copy(dst, pp[:])
                    for st in range(4):
                        ti = ci * 4 + st
                        for dc in range(2):
                            pp = ppool.tile([128, 512], f32, tag="pp", name="pp")
                            for c in range(8):
                                nc.tensor.matmul(
                                    pp[:],
                                    xc[:, c * 512 + st * 128: c * 512 + (st + 1) * 128],
                                    wv_sb[:, c * D + dc * 512: c * D + (dc + 1) * 512],
                                    start=(c == 0), stop=(c == 7))
                            dst = v[:, ti * D + dc * 512: ti * D + (dc + 1) * 512]
                            if (st + dc) % 2 == 0:
                                nc.vector.tensor_copy(dst, pp[:])
                            else:
                                nc.scalar.copy(dst, pp[:])

                # Q^T from full x^T (reuses wk's slot once wk reads are done)
                wq_sb = wpool.tile([128, 8 * D], f16, tag="w0", name="wq_sb")
                for c in range(8):
                    nc.sync.dma_start(wq_sb[:, c * D:(c + 1) * D],
                                      wq[c * 128:(c + 1) * 128, :])
                for ci in range(S // 512):
                    xc = xpool.tile([128, 8 * 512], f16, tag="xc", name="xc")
                    for c in range(8):
                        nc.sync.dma_start(
                            xc[:, c * 512:(c + 1) * 512],
                            xT[c * 128:(c + 1) * 128, ci * 512:(ci + 1) * 512])
                    for m in range(8):
                        pp = ppool.tile([128, 512], f32, tag="pp", name="pp")
                        for c in range(8):
                            nc.tensor.matmul(
                                pp[:],
                                wq_sb[:, c * D + m * 128: c * D + (m + 1) * 128],
                                xc[:, c * 512:(c + 1) * 512],
                                start=(c == 0), stop=(c == 7))
                        dst = qT[:, m * S + ci * 512: m * S + (ci + 1) * 512]
                        if m % 2 == 0:
                            nc.vector.tensor_copy(dst, pp[:])
                        else:
                            nc.scalar.copy(dst, pp[:])

            # ---------------- attention ----------------
            with tc.tile_pool(name="pt", bufs=3) as ptpool, \
                 tc.tile_pool(name="ostg", bufs=3) as ostgpool, \
                 tc.tile_pool(name="lstg", bufs=2) as lstgpool, \
                 tc.tile_pool(name="spsum", bufs=2, space="PSUM") as spool, \
                 tc.tile_pool(name="opsum", bufs=2, space="PSUM") as opool, \
                 tc.tile_pool(name="lpsum", bufs=2, space="PSUM") as lpool:

                def scores(Qb, j):
                    sc = spool.tile([128, QB], f32, tag="sc", name="sc")
                    for c in range(8):
                        nc.tensor.matmul(
                            sc[:],
                            kT[:, c * SP + j * 128: c * SP + (j + 1) * 128],
                            qT[:, c * S + Qb * QB: c * S + (Qb + 1) * QB],
                            start=(c == 0), stop=(c == 7))
                    return sc

                for Qb in range(NQB):
                    nk = Qb + 1   # packed key tiles for this query block
                    ot0 = opool.tile([128, D], f32, tag="ot", name="ot0")
                    ot1 = opool.tile([128, D], f32, tag="ot", name="ot1")
                    lt = lpool.tile([1, QB], f32, tag="lt", name="lt")

                    sc_next = scores(Qb, 0)
                    for j in range(nk):
                        sc = sc_next
                        if j + 1 < nk:
                            sc_next = scores(Qb, j + 1)
                        elif Qb + 1 < NQB:
                            sc_next = scores(Qb + 1, 0)
                        pt = ptpool.tile([128, QB], f16, tag="pt", name="pt")
                        nc.scalar.activation(pt[:], sc[:], Exp, scale=SCALE)
                        if j == nk - 1:
                            nc.vector.tensor_mul(pt[:], pt[:], mask_sb[:])
                        for qt, ot in ((0, ot0), (1, ot1)):
                            ptq = pt[:, qt * 128:(qt + 1) * 128]
                            for dc in range(2):
                                nc.tensor.matmul(
                                    ot[:, dc * 512:(dc + 1) * 512],
                                    ptq,
                                    v[:, j * D + dc * 512: j * D + (dc + 1) * 512],
                                    start=(j == 0), stop=(j == nk - 1))
                        nc.tensor.matmul(lt[:], ones_sb[:], pt[:],
                                         start=(j == 0), stop=(j == nk - 1))

                    og0 = ostgpool.tile([128, D], f32, tag="og", name="og0")
                    nc.vector.tensor_copy(og0[:], ot0[:])
                    nc.sync.dma_start(
                        O[(2 * Qb) * 128:(2 * Qb + 1) * 128, :], og0[:])
                    og1 = ostgpool.tile([128, D], f32, tag="og", name="og1")
                    nc.scalar.copy(og1[:], ot1[:])
                    nc.sync.dma_start(
                        O[(2 * Qb + 1) * 128:(2 * Qb + 2) * 128, :], og1[:])
                    lg = lstgpool.tile([1, QB], f32, tag="lg", name="lg")
                    nc.vector.tensor_copy(lg[:], lt[:])
                    nc.sync.dma_start(L[0:1, Qb * QB:(Qb + 1) * QB], lg[:])

    nc.compile()
    return nc


def _get_program():
    if "nc" not in _PROGRAM_CACHE:
        _PROGRAM_CACHE["nc"] = _build_program()
    return _PROGRAM_CACHE["nc"]


def make_in_maps(x, Wq, Wk, Wv):
    """Host-side prep: cast to fp16, transpose, parity-pack keys, masks."""
    x = np.asarray(x, dtype=np.float32)
    wq16 = np.asarray(Wq, dtype=np.float32).astype(np.float16)
    wk16 = np.asarray(Wk, dtype=np.float32).astype(np.float16)
    wv16 = np.asarray(Wv, dtype=np.float32).astype(np.float16)

    tri = np.triu(np.ones((128, 128), dtype=np.float16))  # allow k<=q
    masks = [
        np.concatenate([tri, np.ones((128, 128), dtype=np.float16)], axis=1),
        np.concatenate([np.zeros((128, 128), dtype=np.float16), tri], axis=1),
    ]

    in_maps = []
    for core in range(N_CORES):
        b, h = divmod(core, 2)
        xb16 = x[b].astype(np.float16)                    # [S, D]
        xT = np.ascontiguousarray(xb16.T)                 # [D, S]
        xp = xb16.reshape(S // 128, 128, D)[h::2].reshape(SP, D)
        xTp = np.ascontiguousarray(xp.T)                  # [D, SP]
        in_maps.append({
            "xT": xT, "xTp": xTp,
            "wq": wq16, "wk": wk16, "wv": wv16,
            "mask": masks[h],
        })
    return in_maps


def combine_outputs(results):
    """results: list of 8 dicts with 'O' [S, D] f32 and 'L' [1, S] f32."""
    out = np.empty((B, S, D), dtype=np.float32)
    for b in range(B):
        O0 = np.asarray(results[2 * b]["O"], dtype=np.float32)
        O1 = np.asarray(results[2 * b + 1]["O"], dtype=np.float32)
        l0 = np.asarray(results[2 * b]["L"], dtype=np.float32).reshape(S)
        l1 = np.asarray(results[2 * b + 1]["L"], dtype=np.float32).reshape(S)
        out[b] = (O0 + O1) / (l0 + l1)[:, None]
    return out


def kernel(x, Wq, Wk, Wv):
    from concourse import bass_utils

    nc = _get_program()
    in_maps = make_in_maps(x, Wq, Wk, Wv)
    res = bass_utils.run_bass_kernel_spmd(nc, in_maps,
                                          core_ids=list(range(N_CORES)))
    return combine_outputs(res.results)


# revision 3
# speedup vs baseline: 2.1792x; 2.1792x over previous
"""Causal single-head attention on 8 TRN2 NeuronCores.

Problem: x [4, 4096, 1024] fp32, Wq/Wk/Wv [1024, 1024] fp32.
  q,k,v = x@W*;  out = softmax(mask(q@k^T)/sqrt(1024)) @ v   per batch.

Sharding: 2 cores per batch (4 batches x 2 = 8 cores). The two cores of a
batch split the KEY dimension by 128-key-tile parity: core h in {0,1} owns
key tiles {h, h+2, h+4, ...}. Every core processes all 4096 queries of its
batch against its ~half of the keys, producing unnormalized partial outputs
  O_h = sum_k exp(s_qk/32) v_k   and   l_h = sum_k exp(s_qk/32)
which the host combines as O = (O_0 + O_1) / (l_0 + l_1).

This parity split makes the per-core program *identical* (SPMD-friendly):
for query block Qb (256 queries = 2 query tiles), both parities process
exactly Qb+1 packed key tiles; the final packed tile is the "diagonal" tile
for one of the parities and either fully-allowed or fully-masked for the
other, handled by one per-core [128, 256] multiplicative mask.

On-device compute uses fp16 matmul inputs (fp32 PSUM accumulation):
fp16 keeps ~10 mantissa bits vs bf16's 8 at identical TensorE throughput.
Softmax skips max-subtraction: logits are ~N(0,1) for this distribution so
exp stays well within fp16/fp32 range (softmax is shift-invariant, so the
result is mathematically identical).
"""

import numpy as np

B, S, D = 4, 4096, 1024
N_CORES = 8
QB = 256            # queries per attention block (2 query tiles)
NQB = S // QB       # 16 blocks
SP = S // 2         # packed keys per core
NKT = SP // 128     # 16 packed key tiles per core
SCALE = 1.0 / 32.0  # 1/sqrt(D_out)

_PROGRAM_CACHE = {}


def _build_program(body_reps=1):
    import concourse.mybir as mybir
    import concourse.tile as tile
    from concourse import bacc

    f16 = mybir.dt.float16
    f32 = mybir.dt.float32

    nc = bacc.Bacc("TRN2", target_bir_lowering=False, debug=False,
                   num_devices=N_CORES)

    xT = nc.dram_tensor("xT", [D, S], f16, kind="ExternalInput").ap()
    xTp = nc.dram_tensor("xTp", [D, SP], f16, kind="ExternalInput").ap()
    wq = nc.dram_tensor("wq", [D, D], f16, kind="ExternalInput").ap()
    wk = nc.dram_tensor("wk", [D, D], f16, kind="ExternalInput").ap()
    wv = nc.dram_tensor("wv", [D, D], f16, kind="ExternalInput").ap()
    mask = nc.dram_tensor("mask", [128, QB], f16, kind="ExternalInput").ap()
    O = nc.dram_tensor("O", [S, D], f32, kind="ExternalOutput").ap()
    L = nc.dram_tensor("L", [1, S], f32, kind="ExternalOutput").ap()

    with tile.TileContext(nc) as tc:
        for _ in range(body_reps):
            _emit_body(nc, tc, xT, xTp, wq, wk, wv, mask, O, L)

    nc.compile()
    return nc


def _emit_body(nc, tc, xT, xTp, wq, wk, wv, mask, O, L):
    import concourse.mybir as mybir

    f16 = mybir.dt.float16
    f32 = mybir.dt.float32
    Exp = mybir.ActivationFunctionType.Exp

    if True:
        with tc.tile_pool(name="res", bufs=1) as res:
            # SBUF-resident projection outputs (layouts: partition x free)
            # kT: K^T packed; d-chunk c lives at cols [c*SP, (c+1)*SP)
            kT = res.tile([128, 8 * SP], f16, tag="kT")
            # v: packed V; key tile j at cols [j*D, (j+1)*D)
            v = res.tile([128, NKT * D], f16, tag="v")
            # qT: Q^T; d-chunk c at cols [c*S, (c+1)*S)
            qT = res.tile([128, 8 * S], f16, tag="qT")
            mask_sb = res.tile([128, QB], f16, tag="mask_sb")
            ones_sb = res.tile([128, 1], f16, tag="ones_sb")
            nc.sync.dma_start(mask_sb[:], mask[:, :])
            nc.vector.memset(ones_sb[:], 1.0)

            # ---------------- QKV projections ----------------
            with tc.tile_pool(name="w", bufs=1) as wpool, \
                 tc.tile_pool(name="xc", bufs=2) as xpool, \
                 tc.tile_pool(name="pproj", bufs=4, space="PSUM") as ppool:
                # W layout: d_in chunk c at cols [c*D, (c+1)*D)
                wk_sb = wpool.tile([128, 8 * D], f16, tag="w0")
                wv_sb = wpool.tile([128, 8 * D], f16, tag="w1")
                for c in range(8):
                    nc.sync.dma_start(wk_sb[:, c * D:(c + 1) * D],
                                      wk[c * 128:(c + 1) * 128, :])
                    nc.sync.dma_start(wv_sb[:, c * D:(c + 1) * D],
                                      wv[c * 128:(c + 1) * 128, :])

                # K^T and V from packed x^T, 512 packed keys per chunk
                for ci in range(SP // 512):
                    xc = xpool.tile([128, 8 * 512], f16, tag="xc")
                    for c in range(8):
                        nc.sync.dma_start(
                            xc[:, c * 512:(c + 1) * 512],
                            xTp[c * 128:(c + 1) * 128, ci * 512:(ci + 1) * 512])
                    for m in range(8):
                        pp = ppool.tile([128, 512], f32, tag="pp", name="pp")
                        for c in range(8):
                            nc.tensor.matmul(
                                pp[:],
                                wk_sb[:, c * D + m * 128: c * D + (m + 1) * 128],
                                xc[:, c * 512:(c + 1) * 512],
                                start=(c == 0), stop=(c == 7))
                        dst = kT[:, m * SP + ci * 512: m * SP + (ci + 1) * 512]
                        if m % 2 == 0:
                            nc.vector.tensor_copy(dst, pp[:])
                        else:
                            nc.scalar.copy(dst, pp[:])
                    for st in range(4):
                        ti = ci * 4 + st
                        for dc in range(2):
                            pp = ppool.tile([128, 512], f32, tag="pp", name="pp")
                            for c in range(8):
                                nc.tensor.matmul(
                                    pp[:],
                                    xc[:, c * 512 + st * 128: c * 512 + (st + 1) * 128],
                                    wv_sb[:, c * D + dc * 512: c * D + (dc + 1) * 512],
                                    start=(c == 0), stop=(c == 7))
                            dst = v[:, ti * D + dc * 512: ti * D + (dc + 1) * 512]
                            if (st + dc) % 2 == 0:
                                nc.vector.tensor_copy(dst, pp[:])
                            else:
                                nc.scalar.copy(dst, pp[:])

                # Q^T from full x^T (reuses wk's slot once wk reads are done)
                wq_sb = wpool.tile([128, 8 * D], f16, tag="w0", name="wq_sb")
                for c in range(8):
                    nc.sync.dma_start(wq_sb[:, c * D:(c + 1) * D],
                                      wq[c * 128:(c + 1) * 128, :])
                for ci in range(S // 512):
                    xc = xpool.tile([128, 8 * 512], f16, tag="xc", name="xc")
                    for c in range(8):
                        nc.sync.dma_start(
                            xc[:, c * 512:(c + 1) * 512],
                            xT[c * 128:(c + 1) * 128, ci * 512:(ci + 1) * 512])
                    for m in range(8):
                        pp = ppool.tile([128, 512], f32, tag="pp", name="pp")
                        for c in range(8):
                            nc.tensor.matmul(
                                pp[:],
                                wq_sb[:, c * D + m * 128: c * D + (m + 1) * 128],
                                xc[:, c * 512:(c + 1) * 512],
                                start=(c == 0), stop=(c == 7))
                        dst = qT[:, m * S + ci * 512: m * S + (ci + 1) * 512]
                        if m % 2 == 0:
                            nc.vector.tensor_copy(dst, pp[:])
                        else:
                            nc.scalar.copy(dst, pp[:])

            # ---------------- attention ----------------
            with tc.tile_pool(name="pt", bufs=3) as ptpool, \
                 tc.tile_pool(name="ostg", bufs=3) as ostgpool, \
                 tc.tile_pool(name="lstg", bufs=2) as lstgpool, \
                 tc.tile_pool(name="spsum", bufs=2, space="PSUM") as spool, \
                 tc.tile_pool(name="opsum", bufs=2, space="PSUM") as opool, \
                 tc.tile_pool(name="lpsum", bufs=2, space="PSUM") as lpool:

                def scores(Qb, j):
                    sc = spool.tile([128, QB], f32, tag="sc", name="sc")
                    for c in range(8):
                        nc.tensor.matmul(
                            sc[:],
                            kT[:, c * SP + j * 128: c * SP + (j + 1) * 128],
                            qT[:, c * S + Qb * QB: c * S + (Qb + 1) * QB],
                            start=(c == 0), stop=(c == 7))
                    return sc

                for Qb in range(NQB):
                    nk = Qb + 1   # packed key tiles for this query block
                    ot0 = opool.tile([128, D], f32, tag="ot", name="ot0")
                    ot1 = opool.tile([128, D], f32, tag="ot", name="ot1")
                    lt = lpool.tile([1, QB], f32, tag="lt", name="lt")

                    sc_next = scores(Qb, 0)
                    for j in range(nk):
                        sc = sc_next
                        if j + 1 < nk:
                            sc_next = scores(Qb, j + 1)
                        elif Qb + 1 < NQB:
                            sc_next = scores(Qb + 1, 0)
                        pt = ptpool.tile([128, QB], f16, tag="pt", name="pt")
                        nc.scalar.activation(pt[:], sc[:], Exp, scale=SCALE)
                        if j == nk - 1:
                            nc.vector.tensor_mul(pt[:], pt[:], mask_sb[:])
                        for qt, ot in ((0, ot0), (1, ot1)):
                            ptq = pt[:, qt * 128:(qt + 1) * 128]
                            for dc in range(2):
                                nc.tensor.matmul(
                                    ot[:, dc * 512:(dc + 1) * 512],
                                    ptq,
                                    v[:, j * D + dc * 512: j * D + (dc + 1) * 512],
                                    start=(j == 0), stop=(j == nk - 1))
                        nc.tensor.matmul(lt[:], ones_sb[:], pt[:],
                                         start=(j == 0), stop=(j == nk - 1))

                    og0 = ostgpool.tile([128, D], f32, tag="og", name="og0")
                    nc.vector.tensor_copy(og0[:], ot0[:])
                    nc.sync.dma_start(
                        O[(2 * Qb) * 128:(2 * Qb + 1) * 128, :], og0[:])
                    og1 = ostgpool.tile([128, D], f32, tag="og", name="og1")
                    nc.scalar.copy(og1[:], ot1[:])
                    nc.sync.dma_start(
                        O[(2 * Qb + 1) * 128:(2 * Qb + 2) * 128, :], og1[:])
                    lg = lstgpool.tile([1, QB], f32, tag="lg", name="lg")
                    nc.vector.tensor_copy(lg[:], lt[:])
                    nc.sync.dma_start(L[0:1, Qb * QB:(Qb + 1) * QB], lg[:])


def _get_program(body_reps=1):
    if body_reps not in _PROGRAM_CACHE:
        _PROGRAM_CACHE[body_reps] = _build_program(body_reps)
    return _PROGRAM_CACHE[body_reps]


def make_in_maps(x, Wq, Wk, Wv):
    """Host-side prep: cast to fp16, transpose, parity-pack keys, masks."""
    x = np.asarray(x, dtype=np.float32)
    wq16 = np.asarray(Wq, dtype=np.float32).astype(np.float16)
    wk16 = np.asarray(Wk, dtype=np.float32).astype(np.float16)
    wv16 = np.asarray(Wv, dtype=np.float32).astype(np.float16)

    tri = np.triu(np.ones((128, 128), dtype=np.float16))  # allow k<=q
    masks = [
        np.concatenate([tri, np.ones((128, 128), dtype=np.float16)], axis=1),
        np.concatenate([np.zeros((128, 128), dtype=np.float16), tri], axis=1),
    ]

    in_maps = []
    for core in range(N_CORES):
        b, h = divmod(core, 2)
        xb16 = x[b].astype(np.float16)                    # [S, D]
        xT = np.ascontiguousarray(xb16.T)                 # [D, S]
        xp = xb16.reshape(S // 128, 128, D)[h::2].reshape(SP, D)
        xTp = np.ascontiguousarray(xp.T)                  # [D, SP]
        in_maps.append({
            "xT": xT, "xTp": xTp,
            "wq": wq16, "wk": wk16, "wv": wv16,
            "mask": masks[h],
        })
    return in_maps


def combine_outputs(results):
    """results: list of 8 dicts with 'O' [S, D] f32 and 'L' [1, S] f32."""
    out = np.empty((B, S, D), dtype=np.float32)
    for b in range(B):
        O0 = np.asarray(results[2 * b]["O"], dtype=np.float32)
        O1 = np.asarray(results[2 * b + 1]["O"], dtype=np.float32)
        l0 = np.asarray(results[2 * b]["L"], dtype=np.float32).reshape(S)
        l1 = np.asarray(results[2 * b + 1]["L"], dtype=np.float32).reshape(S)
        out[b] = (O0 + O1) / (l0 + l1)[:, None]
    return out


def kernel(x, Wq, Wk, Wv):
    from concourse import bass_utils

    nc = _get_program()
    in_maps = make_in_maps(x, Wq, Wk, Wv)
    res = bass_utils.run_bass_kernel_spmd(nc, in_maps,
                                          core_ids=list(range(N_CORES)))
    return combine_outputs(res.results)


# revision 13
# speedup vs baseline: 57.4221x; 26.3505x over previous
"""Causal single-head attention on 8 TRN2 NeuronCores.

Problem: x [4, 4096, 1024] fp32, Wq/Wk/Wv [1024, 1024] fp32.
  q,k,v = x@W*;  out = softmax(mask(q@k^T)/sqrt(1024)) @ v   per batch.

Sharding: 2 cores per batch (4 batches x 2 = 8 cores). The two cores of a
batch split the KEY dimension by 128-key-tile parity: core h in {0,1} owns
key tiles {h, h+2, h+4, ...}. Every core processes all 4096 queries of its
batch against its ~half of the keys, producing unnormalized partial outputs
  O_h = sum_k exp(s_qk/32) v_k   and   l_h = sum_k exp(s_qk/32)
which the host combines as O = (O_0 + O_1) / (l_0 + l_1).

This parity split makes the per-core program *identical* (SPMD-friendly):
for query block Qb (256 queries = 2 query tiles), both parities process
exactly Qb+1 packed key tiles; the final packed tile is the "diagonal" tile
for one of the parities and either fully-allowed or fully-masked for the
other, handled by one per-core [128, 256] multiplicative mask.

On-device compute uses fp16 matmul inputs (fp32 PSUM accumulation):
fp16 keeps ~10 mantissa bits vs bf16's 8 at identical TensorE throughput.
Softmax skips max-subtraction: logits are ~N(0,1) for this distribution so
exp stays well within fp16/fp32 range (softmax is shift-invariant, so the
result is mathematically identical).
"""

import numpy as np

B, S, D = 4, 4096, 1024
N_CORES = 8
QB = 256            # queries per attention block (2 query tiles)
NQB = S // QB       # 16 blocks
SP = S // 2         # packed keys per core
NKT = SP // 128     # 16 packed key tiles per core
SCALE = 1.0 / 32.0  # 1/sqrt(D_out)

_PROGRAM_CACHE = {}


def _build_program(body_reps=1, variant="full"):
    import concourse.mybir as mybir
    import concourse.tile as tile
    from concourse import bacc

    f16 = mybir.dt.float16
    f32 = mybir.dt.float32

    nc = bacc.Bacc("TRN2", target_bir_lowering=False, debug=False,
                   num_devices=N_CORES)

    xT = nc.dram_tensor("xT", [D, S], f16, kind="ExternalInput").ap()
    xTp = nc.dram_tensor("xTp", [D, SP], f16, kind="ExternalInput").ap()
    wq = nc.dram_tensor("wq", [D, D], f16, kind="ExternalInput").ap()
    wk = nc.dram_tensor("wk", [D, D], f16, kind="ExternalInput").ap()
    wv = nc.dram_tensor("wv", [D, D], f16, kind="ExternalInput").ap()
    mask = nc.dram_tensor("mask", [128, QB], f16, kind="ExternalInput").ap()
    O = nc.dram_tensor("O", [S, D], f32, kind="ExternalOutput").ap()
    L = nc.dram_tensor("L", [1, S], f32, kind="ExternalOutput").ap()

    with tile.TileContext(nc) as tc:
        for _ in range(body_reps):
            _emit_body(nc, tc, xT, xTp, wq, wk, wv, mask, O, L,
                       variant=variant)

    nc.compile()
    return nc


def _emit_proj(nc, tc, res, xT, xTp, wq, wk, wv, kT, v, qT):
    import concourse.mybir as mybir
    f16 = mybir.dt.float16
    f32 = mybir.dt.float32

    with tc.tile_pool(name="w", bufs=1) as wpool, \
         tc.tile_pool(name="xc", bufs=2) as xpool, \
         tc.tile_pool(name="pproj", bufs=4, space="PSUM") as ppool:
        # W layout: d_in chunk c at cols [c*D, (c+1)*D)
        wk_sb = wpool.tile([128, 8 * D], f16, tag="w0", name="wk_sb")
        wv_sb = wpool.tile([128, 8 * D], f16, tag="w1", name="wv_sb")
        for c in range(8):
            nc.sync.dma_start(wk_sb[:, c * D:(c + 1) * D],
                              wk[c * 128:(c + 1) * 128, :])
            nc.sync.dma_start(wv_sb[:, c * D:(c + 1) * D],
                              wv[c * 128:(c + 1) * 128, :])

        # K^T and V from packed x^T, 512 packed keys per chunk
        for ci in range(SP // 512):
            xc = xpool.tile([128, 8 * 512], f16, tag="xc", name="xc")
            for c in range(8):
                nc.sync.dma_start(
                    xc[:, c * 512:(c + 1) * 512],
                    xTp[c * 128:(c + 1) * 128, ci * 512:(ci + 1) * 512])
            for m in range(8):
                for hf in range(2):
                    pp = ppool.tile([128, 256], f32, tag="pp", name="pp")
                    for c in range(8):
                        nc.tensor.matmul(
                            pp[:],
                            wk_sb[:, c * D + m * 128: c * D + (m + 1) * 128],
                            xc[:, c * 512 + hf * 256: c * 512 + hf * 256 + 256],
                            start=(c == 0), stop=(c == 7))
                    dst = kT[:, m * SP + ci * 512 + hf * 256:
                             m * SP + ci * 512 + hf * 256 + 256]
                    if (m + hf) % 2 == 0:
                        nc.vector.tensor_copy(dst, pp[:])
                    else:
                        nc.scalar.copy(dst, pp[:])
            for st in range(4):
                ti = ci * 4 + st
                for dc in range(4):
                    pp = ppool.tile([128, 256], f32, tag="pp", name="pp")
                    for c in range(8):
                        nc.tensor.matmul(
                            pp[:],
                            xc[:, c * 512 + st * 128: c * 512 + (st + 1) * 128],
                            wv_sb[:, c * D + dc * 256: c * D + (dc + 1) * 256],
                            start=(c == 0), stop=(c == 7))
                    dst = v[:, ti * D + dc * 256: ti * D + (dc + 1) * 256]
                    if (st + dc) % 2 == 0:
                        nc.vector.tensor_copy(dst, pp[:])
                    else:
                        nc.scalar.copy(dst, pp[:])

        # Q^T from full x^T (reuses wk's slot once wk reads are done)
        wq_sb = wpool.tile([128, 8 * D], f16, tag="w0", name="wq_sb")
        for c in range(8):
            nc.sync.dma_start(wq_sb[:, c * D:(c + 1) * D],
                              wq[c * 128:(c + 1) * 128, :])
        for ci in range(S // 512):
            xc = xpool.tile([128, 8 * 512], f16, tag="xc", name="xc")
            for c in range(8):
                nc.sync.dma_start(
                    xc[:, c * 512:(c + 1) * 512],
                    xT[c * 128:(c + 1) * 128, ci * 512:(ci + 1) * 512])
            for m in range(8):
                for hf in range(2):
                    pp = ppool.tile([128, 256], f32, tag="pp", name="pp")
                    for c in range(8):
                        nc.tensor.matmul(
                            pp[:],
                            wq_sb[:, c * D + m * 128: c * D + (m + 1) * 128],
                            xc[:, c * 512 + hf * 256: c * 512 + hf * 256 + 256],
                            start=(c == 0), stop=(c == 7))
                    dst = qT[:, m * S + ci * 512 + hf * 256:
                             m * S + ci * 512 + hf * 256 + 256]
                    if (m + hf) % 2 == 0:
                        nc.vector.tensor_copy(dst, pp[:])
                    else:
                        nc.scalar.copy(dst, pp[:])


def _emit_attn(nc, tc, res, mask_sb, ones_sb, kT, v, qT, O, L, do_odma):
    import concourse.mybir as mybir
    f16 = mybir.dt.float16
    f32 = mybir.dt.float32
    Exp = mybir.ActivationFunctionType.Exp

    with tc.tile_pool(name="pt", bufs=3) as ptpool, \
         tc.tile_pool(name="ostg", bufs=3) as ostgpool, \
         tc.tile_pool(name="lstg", bufs=2) as lstgpool, \
         tc.tile_pool(name="spsum", bufs=2, space="PSUM") as spool, \
         tc.tile_pool(name="opsum", bufs=2, space="PSUM") as opool, \
         tc.tile_pool(name="lpsum", bufs=2, space="PSUM") as lpool:

        def scores(Qb, j):
            sc = spool.tile([128, QB], f32, tag="sc", name="sc")
            for c in range(8):
                nc.tensor.matmul(
                    sc[:],
                    kT[:, c * SP + j * 128: c * SP + (j + 1) * 128],
                    qT[:, c * S + Qb * QB: c * S + (Qb + 1) * QB],
                    start=(c == 0), stop=(c == 7))
            return sc

        sc_next = scores(0, 0)
        for Qb in range(NQB):
            nk = Qb + 1   # packed key tiles for this query block
            ot0 = opool.tile([128, D], f32, tag="ot", name="ot0")
            ot1 = opool.tile([128, D], f32, tag="ot", name="ot1")
            lt = lpool.tile([1, QB], f32, tag="lt", name="lt")

            for j in range(nk):
                sc = sc_next
                if j + 1 < nk:
                    sc_next = scores(Qb, j + 1)
                elif Qb + 1 < NQB:
                    sc_next = scores(Qb + 1, 0)
                pt = ptpool.tile([128, QB], f16, tag="pt", name="pt")
                nc.scalar.activation(pt[:], sc[:], Exp, scale=SCALE)
                if j == nk - 1:
                    nc.vector.tensor_mul(pt[:], pt[:], mask_sb[:])
                for qt, ot in ((0, ot0), (1, ot1)):
                    ptq = pt[:, qt * 128:(qt + 1) * 128]
                    for dc in range(4):
                        # ot spans 2 PSUM banks; each bank holds two 256-wide
                        # matmul regions, so start/stop go on the first/last
                        # matmul touching the bank (start clears whole bank).
                        nc.tensor.matmul(
                            ot[:, dc * 256:(dc + 1) * 256],
                            ptq,
                            v[:, j * D + dc * 256: j * D + (dc + 1) * 256],
                            start=(j == 0 and dc % 2 == 0),
                            stop=(j == nk - 1 and dc % 2 == 1))
                nc.tensor.matmul(lt[:], ones_sb[:], pt[:],
                                 start=(j == 0), stop=(j == nk - 1))

            og0 = ostgpool.tile([128, D], f32, tag="og", name="og0")
            nc.vector.tensor_copy(og0[:], ot0[:])
            og1 = ostgpool.tile([128, D], f32, tag="og", name="og1")
            nc.scalar.copy(og1[:], ot1[:])
            lg = lstgpool.tile([1, QB], f32, tag="lg", name="lg")
            nc.vector.tensor_copy(lg[:], lt[:])
            if do_odma:
                nc.sync.dma_start(
                    O[(2 * Qb) * 128:(2 * Qb + 1) * 128, :], og0[:])
                nc.sync.dma_start(
                    O[(2 * Qb + 1) * 128:(2 * Qb + 2) * 128, :], og1[:])
                nc.sync.dma_start(L[0:1, Qb * QB:(Qb + 1) * QB], lg[:])


def _emit_body(nc, tc, xT, xTp, wq, wk, wv, mask, O, L, variant="full"):
    import concourse.mybir as mybir
    f16 = mybir.dt.float16

    do_proj = variant in ("full", "proj", "nodma")
    do_attn = variant in ("full", "attn", "nodma")
    do_odma = variant != "nodma"

    with tc.tile_pool(name="res", bufs=1) as res:
        # SBUF-resident projection outputs (layouts: partition x free)
        # kT: K^T packed; d-chunk c lives at cols [c*SP, (c+1)*SP)
        kT = res.tile([128, 8 * SP], f16, tag="kT", name="kT")
        # v: packed V; key tile j at cols [j*D, (j+1)*D)
        v = res.tile([128, NKT * D], f16, tag="v", name="v")
        # qT: Q^T; d-chunk c at cols [c*S, (c+1)*S)
        qT = res.tile([128, 8 * S], f16, tag="qT", name="qT")
        mask_sb = res.tile([128, QB], f16, tag="mask_sb", name="mask_sb")
        ones_sb = res.tile([128, 1], f16, tag="ones_sb", name="ones_sb")
        nc.sync.dma_start(mask_sb[:], mask[:, :])
        nc.vector.memset(ones_sb[:], 1.0)

        if do_proj:
            _emit_proj(nc, tc, res, xT, xTp, wq, wk, wv, kT, v, qT)
        else:
            # timing-only variant: allocate the resident tiles via full
            # memsets so attention reads defined data
            nc.vector.memset(kT[:], 0.25)
            nc.vector.memset(v[:], 0.25)
            nc.vector.memset(qT[:], 0.25)
        if do_attn:
            _emit_attn(nc, tc, res, mask_sb, ones_sb, kT, v, qT, O, L,
                       do_odma)
        if not do_attn:
            # keep outputs written so the NEFF contract stays identical
            og = res.tile([128, D], mybir.dt.float32, tag="og0", name="og")
            nc.vector.tensor_copy(og[:], kT[:, 0:D])
            for qi in range(S // 128):
                nc.sync.dma_start(O[qi * 128:(qi + 1) * 128, :], og[:])
            lg = res.tile([1, S], mybir.dt.float32, tag="lg0", name="lg")
            nc.vector.memset(lg[:], 1.0)
            nc.sync.dma_start(L[:, :], lg[:])


def _get_program(body_reps=1, variant="full"):
    key = (body_reps, variant)
    if key not in _PROGRAM_CACHE:
        _PROGRAM_CACHE[key] = _build_program(body_reps, variant)
    return _PROGRAM_CACHE[key]


def make_in_maps(x, Wq, Wk, Wv):
    """Host-side prep: cast to fp16, transpose, parity-pack keys, masks."""
    x = np.asarray(x, dtype=np.float32)
    wq16 = np.asarray(Wq, dtype=np.float32).astype(np.float16)
    wk16 = np.asarray(Wk, dtype=np.float32).astype(np.float16)
    wv16 = np.asarray(Wv, dtype=np.float32).astype(np.float16)

    tri = np.triu(np.ones((128, 128), dtype=np.float16))  # allow k<=q
    masks = [
        np.concatenate([tri, np.ones((128, 128), dtype=np.float16)], axis=1),
        np.concatenate([np.zeros((128, 128), dtype=np.float16), tri], axis=1),
    ]

    in_maps = []
    for core in range(N_CORES):
        b, h = divmod(core, 2)
        xb16 = x[b].astype(np.float16)                    # [S, D]
        xT = np.ascontiguousarray(xb16.T)                 # [D, S]
        xp = xb16.reshape(S // 128, 128, D)[h::2].reshape(SP, D)
        xTp = np.ascontiguousarray(xp.T)                  # [D, SP]
        in_maps.append({
            "xT": xT, "xTp": xTp,
            "wq": wq16, "wk": wk16, "wv": wv16,
            "mask": masks[h],
        })
    return in_maps


def combine_outputs(results):
    """results: list of 8 dicts with 'O' [S, D] f32 and 'L' [1, S] f32."""
    out = np.empty((B, S, D), dtype=np.float32)
    for b in range(B):
        O0 = np.asarray(results[2 * b]["O"], dtype=np.float32)
        O1 = np.asarray(results[2 * b + 1]["O"], dtype=np.float32)
        l0 = np.asarray(results[2 * b]["L"], dtype=np.float32).reshape(S)
        l1 = np.asarray(results[2 * b + 1]["L"], dtype=np.float32).reshape(S)
        out[b] = (O0 + O1) / (l0 + l1)[:, None]
    return out


def kernel(x, Wq, Wk, Wv):
    from concourse import bass_utils

    nc = _get_program()
    in_maps = make_in_maps(x, Wq, Wk, Wv)
    res = bass_utils.run_bass_kernel_spmd(nc, in_maps,
                                          core_ids=list(range(N_CORES)))
    return combine_outputs(res.results)
